# revision 2
# baseline (speedup 1.0000x reference)
"""GCN (2-layer, 100K nodes) as a Bass/Tile kernel on 8 trn2 cores.

Math: out = A_n @ relu(A_n @ (x@W1) + b1) @ W2 + b2, A_n = D^-1/2 (A+I) D^-1/2.

Sharding: nodes row-sharded across 8 cores; edges partitioned by destination
superblock (256 dst nodes); weights replicated. Transformed features are
all-gathered (collective) so every core can gather arbitrary source rows.

Aggregation: per 128-edge chunk, a one-hot matmul accumulates into PSUM:
    psum[f, dst] += msg[e, f]^T @ mask[e, dst]
    mask[e, :] = (iota256 == dstloc[e]) * norm[e]        (one DVE tensor_scalar)
with norm[e] = dinv[src]*dinv[dst] folding the full symmetric normalization
into the mask. Messages are fetched with dma_gather (fast SWDGE gather,
int16 indices) from the all-gathered table, split into 4 source-index ranges
so indices fit int16. Padding edges gather row 0 and carry norm=0 and
dstloc=511, so they contribute nothing.

The 256-wide moving operand lets fp32 matmuls run in float32r mode
(1 cycle/row instead of 4). Layout chain avoids all transposes except the
tiny [32,128] output transposes in AGG2's epilogue:
  GEMM1: out[n,f]   = xT_blk^T @ W1_blk
  AGG1:  z1T[f,dst] = msg^T @ mask  -> relu(.+b1) (ACT, b1 per-partition)
  GEMM2: out[n,c]   = z1T_blk^T @ W2
  AGG2:  o2T[c,dst] = msg2^T @ mask -> PE-transpose -> out[dst,c]
"""

import sys

sys.path.insert(0, "/opt/trn_rl_repo")

import numpy as np

import concourse.bass as bass
import concourse.bacc as bacc
import concourse.mybir as mybir
import concourse.tile as tile
from concourse.bass_utils import run_bass_kernel_spmd
from concourse.library_config import mlp as _mlp_lib

F32 = mybir.dt.float32
F16 = mybir.dt.float16
I16 = mybir.dt.int16

N_CORES = 8
P = 128
SB = 256          # dst nodes per superblock
NRANGE = 4        # src index ranges (so indices fit int16)
H2P = 128         # h2 feature padding (f16 rows -> 256B-aligned)
GROUP = 4         # superblocks per dma_gather call
NQ = 4            # SWDGE queues (desc-gen parallelism)


def _dims(n_nodes):
    nsb = -(-n_nodes // SB)
    nsb = -(-nsb // N_CORES) * N_CORES      # superblocks, multiple of 8
    np_pad = nsb * SB
    bspc = nsb // N_CORES                   # superblocks per core
    per = bspc * SB                         # rows per core
    return nsb, np_pad, bspc, per


def preprocess(x, edge_index, W1, b1, W2, b2, n_nodes=None):
    n_nodes = n_nodes if n_nodes is not None else x.shape[0]
    in_f = x.shape[1]
    hid = W1.shape[1]
    ncls = W2.shape[1]
    nsb, np_pad, bspc, per = _dims(n_nodes)
    RL = np_pad // NRANGE
    assert RL <= 32767, f"range length {RL} exceeds int16"

    loops = np.arange(n_nodes, dtype=np.int64)
    src = np.concatenate([np.asarray(edge_index[0], dtype=np.int64), loops])
    dst = np.concatenate([np.asarray(edge_index[1], dtype=np.int64), loops])

    deg = np.bincount(dst, minlength=np_pad).astype(np.float32)
    dinv = np.zeros(np_pad, np.float32)
    nz = deg > 0
    dinv[nz] = 1.0 / np.sqrt(deg[nz])
    norm = (dinv[src] * dinv[dst]).astype(np.float32)

    sblk = dst >> 8
    rng = src // RL
    key = sblk * NRANGE + rng
    # sort by src within each (superblock, range) bucket: gather indices
    # become ascending per chunk run -> better HBM row locality
    order = np.lexsort((src, key))
    key_s = key[order]
    counts = np.bincount(key_s, minlength=nsb * NRANGE).reshape(nsb, NRANGE)
    S_r = [int(-(-counts[:, r].max() // P)) for r in range(NRANGE)]
    SC = sum(S_r)
    cum = np.concatenate([[0], np.cumsum(S_r)])  # chunk offsets per range

    # slot position of each edge inside its superblock's SC*128 capacity
    starts = np.zeros(nsb * NRANGE + 1, np.int64)
    starts[1:] = np.cumsum(counts.reshape(-1))
    rank = np.arange(len(order), dtype=np.int64) - starts[key_s]
    col = cum[key_s % NRANGE] * P + rank
    flat = (key_s // NRANGE) * (SC * P) + col

    srcw = np.zeros(nsb * SC * P, np.int16)       # within-range row index
    dstloc = np.full(nsb * SC * P, 2.0 * SB, np.float32)
    normv = np.zeros(nsb * SC * P, np.float32)
    srcw[flat] = (src[order] - rng[order] * RL).astype(np.int16)
    dstloc[flat] = (dst[order] & (SB - 1)).astype(np.float32)
    normv[flat] = norm[order]
    srcw = srcw.reshape(nsb, SC, P)
    # metadata: [nsb, 128, SC]  (lane p of chunk k = edge k*128+p)
    dstl = np.ascontiguousarray(dstloc.reshape(nsb, SC, P).transpose(0, 2, 1))
    nrm = np.ascontiguousarray(normv.reshape(nsb, SC, P).transpose(0, 2, 1))

    # per-range wrapped int16 index arrays: [nsb, 128, S_r*8]
    idx_r = []
    for r in range(NRANGE):
        part = srcw[:, cum[r] : cum[r + 1], :].reshape(nsb, S_r[r] * P)
        wrapped = part.reshape(nsb, S_r[r] * 8, 16).transpose(0, 2, 1)  # [nsb,16,W]
        idx_r.append(np.ascontiguousarray(np.tile(wrapped, (1, 8, 1))))

    xpad = np.zeros((np_pad, in_f), np.float32)
    xpad[:n_nodes] = x
    xT = np.ascontiguousarray(xpad.T)

    W1 = np.ascontiguousarray(W1, dtype=np.float32)
    W2 = np.ascontiguousarray(W2, dtype=np.float32)
    b1c = np.ascontiguousarray(b1.reshape(hid, 1), dtype=np.float32)
    iota = np.tile(np.arange(SB, dtype=np.float16), (P, 1))
    ident = np.eye(P, dtype=np.float32)

    in_maps = []
    for c in range(N_CORES):
        cols = slice(c * per, (c + 1) * per)
        blks = slice(c * bspc, (c + 1) * bspc)
        m = {
            "xT": np.ascontiguousarray(xT[:, cols]),
            "W1": W1,
            "b1": b1c,
            "W2": W2,
            "iota": iota,
            "ident": ident,
            "dstl": np.ascontiguousarray(dstl[blks]),
            "nrm": np.ascontiguousarray(nrm[blks]),
        }
        for r in range(NRANGE):
            # [128, bspc*S_r*8] column-concat of this core's superblocks
            m[f"idx{r}"] = np.ascontiguousarray(
                idx_r[r][blks].transpose(1, 0, 2).reshape(P, bspc * S_r[r] * 8)
            )
        in_maps.append(m)

    meta = dict(
        n_nodes=n_nodes, in_f=in_f, hid=hid, ncls=ncls,
        nsb=nsb, np_pad=np_pad, bspc=bspc, per=per, RL=RL,
        S_r=tuple(S_r), SC=SC,
        b2=np.asarray(b2, dtype=np.float32),
    )
    return in_maps, tuple(S_r), meta


def build_program(S_key, meta, reps=1, timing_variant=False, ablate=()):
    in_f = meta["in_f"]
    hid = meta["hid"]
    ncls = meta["ncls"]
    bspc = meta["bspc"]
    per = meta["per"]
    np_pad = meta["np_pad"]
    RL = meta["RL"]
    S_r = list(meta["S_r"])
    SC = meta["SC"]
    kb_n = in_f // P
    nb_n = per // P
    cum = [0]
    for s in S_r:
        cum.append(cum[-1] + s)

    nc = bacc.Bacc(
        "TRN2", target_bir_lowering=False, debug=False,
        num_devices=1 if timing_variant else N_CORES,
        num_swdge_queues=NQ,
    )

    xT = nc.dram_tensor("xT", [in_f, per], F32, kind="ExternalInput")
    W1 = nc.dram_tensor("W1", [in_f, hid], F32, kind="ExternalInput")
    b1 = nc.dram_tensor("b1", [hid, 1], F32, kind="ExternalInput")
    W2 = nc.dram_tensor("W2", [hid, ncls], F32, kind="ExternalInput")
    iota = nc.dram_tensor("iota", [P, SB], F16, kind="ExternalInput")
    ident = nc.dram_tensor("ident", [P, P], F32, kind="ExternalInput")
    dstl = nc.dram_tensor("dstl", [bspc, P, SC], F32, kind="ExternalInput")
    nrm = nc.dram_tensor("nrm", [bspc, P, SC], F32, kind="ExternalInput")
    idxr = [
        nc.dram_tensor(f"idx{r}", [P, bspc * S_r[r] * 8], I16, kind="ExternalInput")
        for r in range(NRANGE)
    ]
    out = nc.dram_tensor("out", [per, ncls], F32, kind="ExternalOutput")

    groups = [list(range(N_CORES))]
    AL = mybir.AluOpType
    AF = mybir.ActivationFunctionType

    with tile.TileContext(nc) as tc:
        nc.gpsimd.load_library(_mlp_lib)
        with (
            tc.tile_pool(name="const", bufs=1) as const,
            tc.tile_pool(name="dram", bufs=1, space="DRAM") as dram,
            tc.tile_pool(name="xtp", bufs=6) as sb_x,
            tc.tile_pool(name="msgp", bufs=2) as sb_msg,
            tc.tile_pool(name="maskp", bufs=8) as sb_mask,
            tc.tile_pool(name="metap", bufs=4) as sb_meta,
            tc.tile_pool(name="outp", bufs=4) as sb_out,
            tc.tile_pool(name="psum", bufs=3, space="PSUM") as ps,
            tc.tile_pool(name="psum2", bufs=2, space="PSUM") as ps2,
        ):
            w1t = []
            for kb in range(kb_n):
                w = const.tile([P, hid], F32, tag=f"w1_{kb}")
                nc.sync.dma_start(out=w[:], in_=W1[kb * P : (kb + 1) * P, :])
                w1t.append(w)
            w2t = const.tile([P, ncls], F32, tag="w2")
            nc.sync.dma_start(out=w2t[:], in_=W2[:, :])
            b1t = const.tile([P, 1], F32, tag="b1")
            nc.sync.dma_start(out=b1t[:], in_=b1[:, :])
            iot = const.tile([P, SB], F16, tag="iota")
            nc.sync.dma_start(out=iot[:], in_=iota[:, :])
            idt = const.tile([P, P], F32, tag="ident")
            nc.sync.dma_start(out=idt[:], in_=ident[:, :])
            z1T = const.tile([P, per], F32, tag="z1T")

            h_self = dram.tile([per, hid], F16, tag="hself")
            h_full = dram.tile([np_pad, hid], F16, tag="hfull")
            h2_self = dram.tile([per, H2P], F16, tag="h2self")
            h2_full = dram.tile([np_pad, H2P], F16, tag="h2full")

            def agg_phase(layer):
                """Shared AGG loop. layer 1: gather h (elem=hid) -> z1T via
                relu; layer 2: gather h2 (elem=H2P) -> transposed out."""
                table = h_full if layer == 1 else h2_full
                elem = hid if layer == 1 else H2P
                for g0 in range(0, bspc, GROUP):
                    gg = min(GROUP, bspc - g0)
                    msgs = []
                    for r in range(NRANGE):
                        w = S_r[r] * 8
                        idxt = sb_meta.tile([P, gg * w], I16, tag=f"idxt{r}")
                        nc.sync.dma_start(
                            out=idxt[:], in_=idxr[r][:, g0 * w : (g0 + gg) * w]
                        )
                        mt = sb_msg.tile(
                            [P, gg * S_r[r] * elem], F16, tag=f"m_{r}"
                        )
                        nidx = gg * S_r[r] * P
                        if "gather" in ablate:
                            # same bytes via sequential HWDGE DMA
                            nc.sync.dma_start(
                                out=mt[:],
                                in_=table[0 : P * gg * S_r[r], :].rearrange(
                                    "(p k) e -> p (k e)", p=P
                                ),
                            )
                        else:
                            nc.gpsimd.dma_gather(
                                mt[:].rearrange("p (c e) -> p c e", e=elem),
                                table[r * RL : (r + 1) * RL, :],
                                idxt[:],
                                nidx,
                                nidx,
                                elem,
                                single_packet=False,
                                queue_num=r % NQ,
                            )
                        msgs.append(mt)
                    for j in range(gg):
                        sb = g0 + j
                        dstt = sb_meta.tile([P, SC], F32, tag="dstt")
                        nc.sync.dma_start(out=dstt[:], in_=dstl[sb])
                        nrmt = sb_meta.tile([P, SC], F32, tag="nrmt")
                        nc.sync.dma_start(out=nrmt[:], in_=nrm[sb])
                        if layer == 1:
                            acc = ps.tile([P, SB], F32, tag="accA")
                        else:
                            acc = ps2.tile([ncls, SB], F32, tag="accB")
                        k = 0
                        for r in range(NRANGE):
                            for c in range(S_r[r]):
                                mask = sb_mask.tile([P, SB], F16, tag="mask")
                                if "mask" not in ablate:
                                    nc.vector.tensor_scalar(
                                        out=mask[:], in0=iot[:],
                                        scalar1=dstt[:, k : k + 1],
                                        scalar2=nrmt[:, k : k + 1],
                                        op0=AL.is_equal, op1=AL.mult,
                                    )
                                else:
                                    nc.vector.tensor_copy(
                                        out=mask[:, 0:1], in_=iot[:, 0:1]
                                    )
                                base = (j * S_r[r] + c) * elem
                                if layer == 1:
                                    lhs = msgs[r][:, base : base + hid]
                                else:
                                    lhs = msgs[r][:, base : base + ncls]
                                if "mm" not in ablate or k in (0, SC - 1):
                                    nc.tensor.matmul(
                                        acc[:], lhsT=lhs, rhs=mask[:],
                                        start=(k == 0), stop=(k == SC - 1),
                                    )
                                k += 1
                        if layer == 1:
                            nc.scalar.activation(
                                out=z1T[:, sb * SB : (sb + 1) * SB], in_=acc[:],
                                func=AF.Relu, bias=b1t[:, 0:1], scale=1.0,
                            )
                        else:
                            o2t = sb_out.tile([ncls, SB], F32, tag="o2t")
                            nc.scalar.copy(out=o2t[:], in_=acc[:])
                            for half in range(SB // P):
                                ptr = ps2.tile([P, ncls], F32, tag="ptp")
                                nc.tensor.transpose(
                                    ptr[:],
                                    o2t[:, half * P : (half + 1) * P],
                                    idt[:ncls, :ncls],
                                )
                                osb = sb_out.tile([P, ncls], F32, tag="osb")
                                nc.vector.tensor_copy(out=osb[:], in_=ptr[:])
                                row = sb * SB + half * P
                                nc.sync.dma_start(
                                    out=out[row : row + P, :], in_=osb[:]
                                )

            for _rep in range(reps):
                # ---- GEMM1: h = x @ W1 (own shard) ----
                for nb in range(nb_n):
                    ph = ps.tile([P, hid], F32, tag="accA")
                    for kb in range(kb_n):
                        xt = sb_x.tile([P, P], F32, tag="xt")
                        nc.sync.dma_start(
                            out=xt[:],
                            in_=xT[kb * P : (kb + 1) * P, nb * P : (nb + 1) * P],
                        )
                        nc.tensor.matmul(
                            ph[:], lhsT=xt[:], rhs=w1t[kb][:],
                            start=(kb == 0), stop=(kb == kb_n - 1),
                        )
                    hsb = sb_out.tile([P, hid], F16, tag="hsb")
                    nc.scalar.copy(out=hsb[:], in_=ph[:])
                    nc.sync.dma_start(
                        out=h_self[nb * P : (nb + 1) * P, :], in_=hsb[:]
                    )

                if timing_variant or "cc" in ablate:
                    nc.sync.dma_start(out=h_full[0:per, :], in_=h_self[:])
                else:
                    nc.gpsimd.collective_compute(
                        "AllGather", mybir.AluOpType.bypass, replica_groups=groups,
                        ins=[h_self.opt()], outs=[h_full.opt()],
                    )

                agg_phase(layer=1)

                # ---- GEMM2: h2 = z1 @ W2 (own shard) ----
                for nb in range(nb_n):
                    p2 = ps.tile([P, ncls], F32, tag="accA")
                    nc.tensor.matmul(
                        p2[:], lhsT=z1T[:, nb * P : (nb + 1) * P], rhs=w2t[:],
                        start=True, stop=True,
                    )
                    h2sb = sb_out.tile([P, ncls], F16, tag="h2sb")
                    nc.scalar.copy(out=h2sb[:], in_=p2[:])
                    nc.sync.dma_start(
                        out=h2_self[nb * P : (nb + 1) * P, :ncls], in_=h2sb[:]
                    )

                if timing_variant or "cc" in ablate:
                    nc.sync.dma_start(out=h2_full[0:per, :], in_=h2_self[:])
                else:
                    nc.gpsimd.collective_compute(
                        "AllGather", mybir.AluOpType.bypass, replica_groups=groups,
                        ins=[h2_self.opt()], outs=[h2_full.opt()],
                    )

                agg_phase(layer=2)

    nc.compile()
    return nc


_CACHE = {}


def get_program(S_key, meta, reps=1):
    key = (S_key, meta["n_nodes"], meta["in_f"], reps)
    if key not in _CACHE:
        _CACHE[key] = build_program(S_key, meta, reps=reps)
    return _CACHE[key]


def assemble(results, meta):
    n_nodes = meta["n_nodes"]
    ncls = meta["ncls"]
    full = np.concatenate([results[c]["out"] for c in range(N_CORES)], axis=0)
    return full[:n_nodes].astype(np.float32) + meta["b2"].reshape(1, ncls)


def kernel(x, edge_index, W1, b1, W2, b2):
    x = np.asarray(x)
    edge_index = np.asarray(edge_index)
    in_maps, S_key, meta = preprocess(x, edge_index, W1, b1, W2, b2)
    nc = get_program(S_key, meta, reps=1)
    res = run_bass_kernel_spmd(nc, in_maps, list(range(N_CORES)))
    return assemble(res.results, meta)


# ---------------------------------------------------------------------------
# Benchmarking helpers (used by test.py; not needed for grading correctness).
# The axon PJRT round trip has ~90-120ms noise, so HW time is estimated from
# the marginal cost of replicating the kernel body inside one NEFF.
# ---------------------------------------------------------------------------


def _make_runner(nc, in_maps):
    import jax
    from jax.sharding import Mesh, PartitionSpec
    from jax.experimental.shard_map import shard_map
    from concourse import bass2jax

    bass2jax.install_neuronx_cc_hook()
    partition_name = nc.partition_id_tensor.name if nc.partition_id_tensor else None
    in_names, out_names, out_avals, zero_outs = [], [], [], []
    for alloc in nc.m.functions[0].allocations:
        if not isinstance(alloc, mybir.MemoryLocationSet):
            continue
        name = alloc.memorylocations[0].name
        if alloc.kind == "ExternalInput":
            if name != partition_name:
                in_names.append(name)
        elif alloc.kind == "ExternalOutput":
            out_names.append(name)
            shape = tuple(alloc.tensor_shape)
            dtype = mybir.dt.np(alloc.dtype)
            out_avals.append(jax.core.ShapedArray(shape, dtype))
            zero_outs.append(np.zeros(shape, dtype))
    n_params = len(in_names)
    all_in = in_names + out_names + ([partition_name] if partition_name else [])

    def _body(*args):
        operands = list(args)
        if partition_name is not None:
            operands.append(bass2jax.partition_id_tensor())
        outs = bass2jax._bass_exec_p.bind(
            *operands,
            out_avals=tuple(out_avals),
            in_names=tuple(all_in),
            out_names=tuple(out_names),
            lowering_input_output_aliases=(),
            sim_require_finite=False,
            sim_require_nnan=False,
            nc=nc,
        )
        return tuple(outs)

    devices = jax.devices()[:N_CORES]
    mesh = Mesh(np.asarray(devices), ("core",))
    n_outs = len(out_names)
    fn = jax.jit(
        shard_map(
            _body,
            mesh=mesh,
            in_specs=(PartitionSpec("core"),) * (n_params + n_outs),
            out_specs=(PartitionSpec("core"),) * n_outs,
            check_rep=False,
        ),
        keep_unused=True,
    )
    concat_in = [
        np.concatenate([np.asarray(in_maps[c][n]) for c in range(N_CORES)], axis=0)
        for n in in_names
    ]
    concat_zero = [
        np.zeros((N_CORES * z.shape[0], *z.shape[1:]), z.dtype) for z in zero_outs
    ]
    args = [jax.device_put(a) for a in concat_in + concat_zero]

    def run():
        outs = fn(*args)
        jax.block_until_ready(outs)
        return outs

    return run, out_names, out_avals


def _time_runner(run, iters=8):
    import time

    run()
    best = float("inf")
    for _ in range(iters):
        t0 = time.perf_counter()
        run()
        best = min(best, time.perf_counter() - t0)
    return best


def bench_hw_ns(in_maps, S_key, meta, reps_list=(1, 3, 5), iters=10):
    """Min-wall slope across in-NEFF repetition counts."""
    import time

    runners = []
    for r in reps_list:
        nc = get_program(S_key, meta, reps=r)
        run, _, _ = _make_runner(nc, in_maps)
        for _ in range(3):
            run()
        runners.append(run)
    times = {r: [] for r in reps_list}
    for _ in range(iters):
        for r, run in zip(reps_list, runners):
            t0 = time.perf_counter()
            run()
            times[r].append(time.perf_counter() - t0)
    mins = [min(times[r]) * 1e3 for r in reps_list]
    for r, m in zip(reps_list, mins):
        print(f"  reps={r}: min wall {m:.2f} ms")
    slope = (mins[-1] - mins[0]) / (reps_list[-1] - reps_list[0])
    return slope * 1e6



# revision 3
# speedup vs baseline: 6.0308x; 6.0308x over previous
"""GCN (2-layer, 100K nodes) as a Bass/Tile kernel on 8 trn2 cores.

Math: out = A_n @ relu(A_n @ (x@W1) + b1) @ W2 + b2, A_n = D^-1/2 (A+I) D^-1/2.

Sharding: nodes row-sharded across 8 cores; edges partitioned by destination
superblock (256 dst nodes); weights replicated. Transformed features are
all-gathered (collective) so every core can gather arbitrary source rows.

Aggregation: per 128-edge chunk, a one-hot matmul accumulates into PSUM:
    psum[f, dst] += msg[e, f]^T @ mask[e, dst]
    mask[e, :] = (iota256 == dstloc[e]) * norm[e]        (one DVE tensor_scalar)
with norm[e] = dinv[src]*dinv[dst] folding the full symmetric normalization
into the mask. Messages are fetched with dma_gather (fast SWDGE gather,
int16 indices) from the all-gathered table, split into 4 source-index ranges
so indices fit int16. Padding edges gather row 0 and carry norm=0 and
dstloc=511, so they contribute nothing.

The 256-wide moving operand lets fp32 matmuls run in float32r mode
(1 cycle/row instead of 4). Layout chain avoids all transposes except the
tiny [32,128] output transposes in AGG2's epilogue:
  GEMM1: out[n,f]   = xT_blk^T @ W1_blk
  AGG1:  z1T[f,dst] = msg^T @ mask  -> relu(.+b1) (ACT, b1 per-partition)
  GEMM2: out[n,c]   = z1T_blk^T @ W2
  AGG2:  o2T[c,dst] = msg2^T @ mask -> PE-transpose -> out[dst,c]
"""

import sys

sys.path.insert(0, "/opt/trn_rl_repo")

import numpy as np

import concourse.bass as bass
import concourse.bacc as bacc
import concourse.mybir as mybir
import concourse.tile as tile
from concourse.bass_utils import run_bass_kernel_spmd
from concourse.library_config import mlp as _mlp_lib

F32 = mybir.dt.float32
F16 = mybir.dt.float16
I16 = mybir.dt.int16

N_CORES = 8
P = 128
SB = 256          # dst nodes per superblock
NRANGE = 4        # src index ranges (so indices fit int16)
H2P = 128         # h2 feature padding (f16 rows -> 256B-aligned)
GROUP = 4         # superblocks per dma_gather call
NQ = 4            # SWDGE queues (desc-gen parallelism)


def _dims(n_nodes):
    nsb = -(-n_nodes // SB)
    nsb = -(-nsb // N_CORES) * N_CORES      # superblocks, multiple of 8
    np_pad = nsb * SB
    bspc = nsb // N_CORES                   # superblocks per core
    per = bspc * SB                         # rows per core
    return nsb, np_pad, bspc, per


def preprocess(x, edge_index, W1, b1, W2, b2, n_nodes=None):
    n_nodes = n_nodes if n_nodes is not None else x.shape[0]
    in_f = x.shape[1]
    hid = W1.shape[1]
    ncls = W2.shape[1]
    nsb, np_pad, bspc, per = _dims(n_nodes)
    RL = np_pad // NRANGE
    assert RL <= 32767, f"range length {RL} exceeds int16"

    loops = np.arange(n_nodes, dtype=np.int64)
    src = np.concatenate([np.asarray(edge_index[0], dtype=np.int64), loops])
    dst = np.concatenate([np.asarray(edge_index[1], dtype=np.int64), loops])

    deg = np.bincount(dst, minlength=np_pad).astype(np.float32)
    dinv = np.zeros(np_pad, np.float32)
    nz = deg > 0
    dinv[nz] = 1.0 / np.sqrt(deg[nz])
    norm = (dinv[src] * dinv[dst]).astype(np.float32)

    sblk = dst >> 8
    rng = src // RL
    key = sblk * NRANGE + rng
    order = np.argsort(key, kind="stable")
    key_s = key[order]
    counts = np.bincount(key_s, minlength=nsb * NRANGE).reshape(nsb, NRANGE)
    S_r = [int(-(-counts[:, r].max() // P)) for r in range(NRANGE)]
    SC = sum(S_r)
    cum = np.concatenate([[0], np.cumsum(S_r)])  # chunk offsets per range

    # slot position of each edge inside its superblock's SC*128 capacity
    starts = np.zeros(nsb * NRANGE + 1, np.int64)
    starts[1:] = np.cumsum(counts.reshape(-1))
    rank = np.arange(len(order), dtype=np.int64) - starts[key_s]
    col = cum[key_s % NRANGE] * P + rank
    flat = (key_s // NRANGE) * (SC * P) + col

    srcw = np.zeros(nsb * SC * P, np.int16)       # within-range row index
    dstloc = np.full(nsb * SC * P, 2.0 * SB, np.float32)
    normv = np.zeros(nsb * SC * P, np.float32)
    srcw[flat] = (src[order] - rng[order] * RL).astype(np.int16)
    dstloc[flat] = (dst[order] & (SB - 1)).astype(np.float32)
    normv[flat] = norm[order]
    srcw = srcw.reshape(nsb, SC, P)
    # metadata: [nsb, 128, SC]  (lane p of chunk k = edge k*128+p)
    dstl = np.ascontiguousarray(dstloc.reshape(nsb, SC, P).transpose(0, 2, 1))
    nrm = np.ascontiguousarray(normv.reshape(nsb, SC, P).transpose(0, 2, 1))

    # per-range wrapped int16 index arrays: [nsb, 128, S_r*8]
    idx_r = []
    for r in range(NRANGE):
        part = srcw[:, cum[r] : cum[r + 1], :].reshape(nsb, S_r[r] * P)
        wrapped = part.reshape(nsb, S_r[r] * 8, 16).transpose(0, 2, 1)  # [nsb,16,W]
        idx_r.append(np.ascontiguousarray(np.tile(wrapped, (1, 8, 1))))

    xpad = np.zeros((np_pad, in_f), np.float32)
    xpad[:n_nodes] = x
    xT = np.ascontiguousarray(xpad.T)

    W1 = np.ascontiguousarray(W1, dtype=np.float32)
    W2 = np.ascontiguousarray(W2, dtype=np.float32)
    b1c = np.ascontiguousarray(b1.reshape(hid, 1), dtype=np.float32)
    iota = np.tile(np.arange(SB, dtype=np.float16), (P, 1))
    ident = np.eye(P, dtype=np.float32)

    in_maps = []
    for c in range(N_CORES):
        cols = slice(c * per, (c + 1) * per)
        blks = slice(c * bspc, (c + 1) * bspc)
        m = {
            "xT": np.ascontiguousarray(xT[:, cols]),
            "W1": W1,
            "b1": b1c,
            "W2": W2,
            "iota": iota,
            "ident": ident,
            "dstl": np.ascontiguousarray(dstl[blks]),
            "nrm": np.ascontiguousarray(nrm[blks]),
        }
        for r in range(NRANGE):
            # [128, bspc*S_r*8] column-concat of this core's superblocks
            m[f"idx{r}"] = np.ascontiguousarray(
                idx_r[r][blks].transpose(1, 0, 2).reshape(P, bspc * S_r[r] * 8)
            )
        in_maps.append(m)

    meta = dict(
        n_nodes=n_nodes, in_f=in_f, hid=hid, ncls=ncls,
        nsb=nsb, np_pad=np_pad, bspc=bspc, per=per, RL=RL,
        S_r=tuple(S_r), SC=SC,
        b2=np.asarray(b2, dtype=np.float32),
    )
    return in_maps, tuple(S_r), meta


def build_program(S_key, meta, reps=1, timing_variant=False, ablate=()):
    in_f = meta["in_f"]
    hid = meta["hid"]
    ncls = meta["ncls"]
    bspc = meta["bspc"]
    per = meta["per"]
    np_pad = meta["np_pad"]
    RL = meta["RL"]
    S_r = list(meta["S_r"])
    SC = meta["SC"]
    kb_n = in_f // P
    nb_n = per // P
    cum = [0]
    for s in S_r:
        cum.append(cum[-1] + s)

    nc = bacc.Bacc(
        "TRN2", target_bir_lowering=False, debug=False,
        num_devices=1 if timing_variant else N_CORES,
        num_swdge_queues=NQ,
    )

    xT = nc.dram_tensor("xT", [in_f, per], F32, kind="ExternalInput")
    W1 = nc.dram_tensor("W1", [in_f, hid], F32, kind="ExternalInput")
    b1 = nc.dram_tensor("b1", [hid, 1], F32, kind="ExternalInput")
    W2 = nc.dram_tensor("W2", [hid, ncls], F32, kind="ExternalInput")
    iota = nc.dram_tensor("iota", [P, SB], F16, kind="ExternalInput")
    ident = nc.dram_tensor("ident", [P, P], F32, kind="ExternalInput")
    dstl = nc.dram_tensor("dstl", [bspc, P, SC], F32, kind="ExternalInput")
    nrm = nc.dram_tensor("nrm", [bspc, P, SC], F32, kind="ExternalInput")
    idxr = [
        nc.dram_tensor(f"idx{r}", [P, bspc * S_r[r] * 8], I16, kind="ExternalInput")
        for r in range(NRANGE)
    ]
    out = nc.dram_tensor("out", [per, ncls], F32, kind="ExternalOutput")

    groups = [list(range(N_CORES))]
    AL = mybir.AluOpType
    AF = mybir.ActivationFunctionType

    with tile.TileContext(nc) as tc:
        nc.gpsimd.load_library(_mlp_lib)
        with (
            tc.tile_pool(name="const", bufs=1) as const,
            tc.tile_pool(name="dram", bufs=1, space="DRAM") as dram,
            tc.tile_pool(name="xtp", bufs=6) as sb_x,
            tc.tile_pool(name="msgp", bufs=2) as sb_msg,
            tc.tile_pool(name="maskp", bufs=8) as sb_mask,
            tc.tile_pool(name="metap", bufs=4) as sb_meta,
            tc.tile_pool(name="outp", bufs=4) as sb_out,
            tc.tile_pool(name="psum", bufs=3, space="PSUM") as ps,
            tc.tile_pool(name="psum2", bufs=2, space="PSUM") as ps2,
        ):
            w1t = []
            for kb in range(kb_n):
                w = const.tile([P, hid], F32, tag=f"w1_{kb}")
                nc.sync.dma_start(out=w[:], in_=W1[kb * P : (kb + 1) * P, :])
                w1t.append(w)
            w2t = const.tile([P, ncls], F32, tag="w2")
            nc.sync.dma_start(out=w2t[:], in_=W2[:, :])
            b1t = const.tile([P, 1], F32, tag="b1")
            nc.sync.dma_start(out=b1t[:], in_=b1[:, :])
            iot = const.tile([P, SB], F16, tag="iota")
            nc.sync.dma_start(out=iot[:], in_=iota[:, :])
            idt = const.tile([P, P], F32, tag="ident")
            nc.sync.dma_start(out=idt[:], in_=ident[:, :])
            z1T = const.tile([P, per], F32, tag="z1T")

            h_self = dram.tile([per, hid], F16, tag="hself")
            h_full = dram.tile([np_pad, hid], F16, tag="hfull")
            h2_self = dram.tile([per, H2P], F16, tag="h2self")
            h2_full = dram.tile([np_pad, H2P], F16, tag="h2full")

            def agg_phase(layer):
                """Shared AGG loop. layer 1: gather h (elem=hid) -> z1T via
                relu; layer 2: gather h2 (elem=H2P) -> transposed out."""
                table = h_full if layer == 1 else h2_full
                elem = hid if layer == 1 else H2P
                for g0 in range(0, bspc, GROUP):
                    gg = min(GROUP, bspc - g0)
                    msgs = []
                    for r in range(NRANGE):
                        w = S_r[r] * 8
                        idxt = sb_meta.tile([P, gg * w], I16, tag=f"idxt{r}")
                        nc.sync.dma_start(
                            out=idxt[:], in_=idxr[r][:, g0 * w : (g0 + gg) * w]
                        )
                        mt = sb_msg.tile(
                            [P, gg * S_r[r] * elem], F16, tag=f"m_{r}"
                        )
                        nidx = gg * S_r[r] * P
                        if "gather" in ablate:
                            # same bytes via sequential HWDGE DMA
                            nc.sync.dma_start(
                                out=mt[:],
                                in_=table[0 : P * gg * S_r[r], :].rearrange(
                                    "(p k) e -> p (k e)", p=P
                                ),
                            )
                        else:
                            nc.gpsimd.dma_gather(
                                mt[:].rearrange("p (c e) -> p c e", e=elem),
                                table[r * RL : (r + 1) * RL, :],
                                idxt[:],
                                nidx,
                                nidx,
                                elem,
                                single_packet=False,
                                queue_num=r % NQ,
                            )
                        msgs.append(mt)
                    for j in range(gg):
                        sb = g0 + j
                        dstt = sb_meta.tile([P, SC], F32, tag="dstt")
                        nc.sync.dma_start(out=dstt[:], in_=dstl[sb])
                        nrmt = sb_meta.tile([P, SC], F32, tag="nrmt")
                        nc.sync.dma_start(out=nrmt[:], in_=nrm[sb])
                        if layer == 1:
                            acc = ps.tile([P, SB], F32, tag="accA")
                        else:
                            acc = ps2.tile([ncls, SB], F32, tag="accB")
                        k = 0
                        for r in range(NRANGE):
                            for c in range(S_r[r]):
                                mask = sb_mask.tile([P, SB], F16, tag="mask")
                                if "mask" not in ablate:
                                    nc.vector.tensor_scalar(
                                        out=mask[:], in0=iot[:],
                                        scalar1=dstt[:, k : k + 1],
                                        scalar2=nrmt[:, k : k + 1],
                                        op0=AL.is_equal, op1=AL.mult,
                                    )
                                else:
                                    nc.vector.tensor_copy(
                                        out=mask[:, 0:1], in_=iot[:, 0:1]
                                    )
                                base = (j * S_r[r] + c) * elem
                                if layer == 1:
                                    lhs = msgs[r][:, base : base + hid]
                                else:
                                    lhs = msgs[r][:, base : base + ncls]
                                if "mm" not in ablate or k in (0, SC - 1):
                                    nc.tensor.matmul(
                                        acc[:], lhsT=lhs, rhs=mask[:],
                                        start=(k == 0), stop=(k == SC - 1),
                                    )
                                k += 1
                        if layer == 1:
                            nc.scalar.activation(
                                out=z1T[:, sb * SB : (sb + 1) * SB], in_=acc[:],
                                func=AF.Relu, bias=b1t[:, 0:1], scale=1.0,
                            )
                        else:
                            o2t = sb_out.tile([ncls, SB], F32, tag="o2t")
                            nc.scalar.copy(out=o2t[:], in_=acc[:])
                            for half in range(SB // P):
                                ptr = ps2.tile([P, ncls], F32, tag="ptp")
                                nc.tensor.transpose(
                                    ptr[:],
                                    o2t[:, half * P : (half + 1) * P],
                                    idt[:ncls, :ncls],
                                )
                                osb = sb_out.tile([P, ncls], F32, tag="osb")
                                nc.vector.tensor_copy(out=osb[:], in_=ptr[:])
                                row = sb * SB + half * P
                                nc.sync.dma_start(
                                    out=out[row : row + P, :], in_=osb[:]
                                )

            for _rep in range(reps):
                # ---- GEMM1: h = x @ W1 (own shard) ----
                for nb in range(nb_n):
                    ph = ps.tile([P, hid], F32, tag="accA")
                    for kb in range(kb_n):
                        xt = sb_x.tile([P, P], F32, tag="xt")
                        nc.sync.dma_start(
                            out=xt[:],
                            in_=xT[kb * P : (kb + 1) * P, nb * P : (nb + 1) * P],
                        )
                        nc.tensor.matmul(
                            ph[:], lhsT=xt[:], rhs=w1t[kb][:],
                            start=(kb == 0), stop=(kb == kb_n - 1),
                        )
                    hsb = sb_out.tile([P, hid], F16, tag="hsb")
                    nc.scalar.copy(out=hsb[:], in_=ph[:])
                    nc.sync.dma_start(
                        out=h_self[nb * P : (nb + 1) * P, :], in_=hsb[:]
                    )

                if timing_variant or "cc" in ablate:
                    nc.sync.dma_start(out=h_full[0:per, :], in_=h_self[:])
                else:
                    nc.gpsimd.collective_compute(
                        "AllGather", mybir.AluOpType.bypass, replica_groups=groups,
                        ins=[h_self.opt()], outs=[h_full.opt()],
                    )

                agg_phase(layer=1)

                # ---- GEMM2: h2 = z1 @ W2 (own shard) ----
                for nb in range(nb_n):
                    p2 = ps.tile([P, ncls], F32, tag="accA")
                    nc.tensor.matmul(
                        p2[:], lhsT=z1T[:, nb * P : (nb + 1) * P], rhs=w2t[:],
                        start=True, stop=True,
                    )
                    h2sb = sb_out.tile([P, ncls], F16, tag="h2sb")
                    nc.scalar.copy(out=h2sb[:], in_=p2[:])
                    nc.sync.dma_start(
                        out=h2_self[nb * P : (nb + 1) * P, :ncls], in_=h2sb[:]
                    )

                if timing_variant or "cc" in ablate:
                    nc.sync.dma_start(out=h2_full[0:per, :], in_=h2_self[:])
                else:
                    nc.gpsimd.collective_compute(
                        "AllGather", mybir.AluOpType.bypass, replica_groups=groups,
                        ins=[h2_self.opt()], outs=[h2_full.opt()],
                    )

                agg_phase(layer=2)

    nc.compile()
    return nc


_CACHE = {}


def get_program(S_key, meta, reps=1):
    key = (S_key, meta["n_nodes"], meta["in_f"], reps)
    if key not in _CACHE:
        _CACHE[key] = build_program(S_key, meta, reps=reps)
    return _CACHE[key]


def assemble(results, meta):
    n_nodes = meta["n_nodes"]
    ncls = meta["ncls"]
    full = np.concatenate([results[c]["out"] for c in range(N_CORES)], axis=0)
    return full[:n_nodes].astype(np.float32) + meta["b2"].reshape(1, ncls)


def kernel(x, edge_index, W1, b1, W2, b2):
    x = np.asarray(x)
    edge_index = np.asarray(edge_index)
    in_maps, S_key, meta = preprocess(x, edge_index, W1, b1, W2, b2)
    nc = get_program(S_key, meta, reps=1)
    res = run_bass_kernel_spmd(nc, in_maps, list(range(N_CORES)))
    return assemble(res.results, meta)


# ---------------------------------------------------------------------------
# Benchmarking helpers (used by test.py; not needed for grading correctness).
# The axon PJRT round trip has ~90-120ms noise, so HW time is estimated from
# the marginal cost of replicating the kernel body inside one NEFF.
# ---------------------------------------------------------------------------


def _make_runner(nc, in_maps):
    import jax
    from jax.sharding import Mesh, PartitionSpec
    from jax.experimental.shard_map import shard_map
    from concourse import bass2jax

    bass2jax.install_neuronx_cc_hook()
    partition_name = nc.partition_id_tensor.name if nc.partition_id_tensor else None
    in_names, out_names, out_avals, zero_outs = [], [], [], []
    for alloc in nc.m.functions[0].allocations:
        if not isinstance(alloc, mybir.MemoryLocationSet):
            continue
        name = alloc.memorylocations[0].name
        if alloc.kind == "ExternalInput":
            if name != partition_name:
                in_names.append(name)
        elif alloc.kind == "ExternalOutput":
            out_names.append(name)
            shape = tuple(alloc.tensor_shape)
            dtype = mybir.dt.np(alloc.dtype)
            out_avals.append(jax.core.ShapedArray(shape, dtype))
            zero_outs.append(np.zeros(shape, dtype))
    n_params = len(in_names)
    all_in = in_names + out_names + ([partition_name] if partition_name else [])

    def _body(*args):
        operands = list(args)
        if partition_name is not None:
            operands.append(bass2jax.partition_id_tensor())
        outs = bass2jax._bass_exec_p.bind(
            *operands,
            out_avals=tuple(out_avals),
            in_names=tuple(all_in),
            out_names=tuple(out_names),
            lowering_input_output_aliases=(),
            sim_require_finite=False,
            sim_require_nnan=False,
            nc=nc,
        )
        return tuple(outs)

    devices = jax.devices()[:N_CORES]
    mesh = Mesh(np.asarray(devices), ("core",))
    n_outs = len(out_names)
    fn = jax.jit(
        shard_map(
            _body,
            mesh=mesh,
            in_specs=(PartitionSpec("core"),) * (n_params + n_outs),
            out_specs=(PartitionSpec("core"),) * n_outs,
            check_rep=False,
        ),
        keep_unused=True,
    )
    concat_in = [
        np.concatenate([np.asarray(in_maps[c][n]) for c in range(N_CORES)], axis=0)
        for n in in_names
    ]
    concat_zero = [
        np.zeros((N_CORES * z.shape[0], *z.shape[1:]), z.dtype) for z in zero_outs
    ]
    args = [jax.device_put(a) for a in concat_in + concat_zero]

    def run():
        outs = fn(*args)
        jax.block_until_ready(outs)
        return outs

    return run, out_names, out_avals


def _time_runner(run, iters=8):
    import time

    run()
    best = float("inf")
    for _ in range(iters):
        t0 = time.perf_counter()
        run()
        best = min(best, time.perf_counter() - t0)
    return best


def bench_hw_ns(in_maps, S_key, meta, reps_list=(1, 3, 5), iters=10):
    """Min-wall slope across in-NEFF repetition counts."""
    import time

    runners = []
    for r in reps_list:
        nc = get_program(S_key, meta, reps=r)
        run, _, _ = _make_runner(nc, in_maps)
        for _ in range(3):
            run()
        runners.append(run)
    times = {r: [] for r in reps_list}
    for _ in range(iters):
        for r, run in zip(reps_list, runners):
            t0 = time.perf_counter()
            run()
            times[r].append(time.perf_counter() - t0)
    mins = [min(times[r]) * 1e3 for r in reps_list]
    for r, m in zip(reps_list, mins):
        print(f"  reps={r}: min wall {m:.2f} ms")
    slope = (mins[-1] - mins[0]) / (reps_list[-1] - reps_list[0])
    return slope * 1e6

